# revision 1
# baseline (speedup 1.0000x reference)
# Multi-headed attention (B=2, A=6, S=1024, E=256, d_model=512, H=8, DK=64)
# distributed over 8 NeuronCores.
#
# Decomposition: the 12 (batch, agent) pairs are each split into two
# "quad-tasks" of 4 heads (d_model halves), giving 24 tasks; each core runs
# 3 tasks (perfect balance, no duplicated FLOPs: QKV projections split
# cleanly along the head dim, the output projection's head contraction is
# summed on the host).
#
# Per-task device pipeline (no on-device transposes anywhere):
#   inputs arrive host-pre-transposed as x^T [E, S].
#   QT = Wq_t^T @ q^T   [F=256, S]      (lhsT = Wq_t, rhs = q^T)
#   KT = Wk_t^T @ k^T   [F=256, S]
#   V  = (v^T)^T @ Wv_t [S, F]          (lhsT = v^T slice, rhs = Wv_t)
#   per head h (64 rows of QT/KT):
#     scoresT[k, q] = K_h @ Q_h^T       (lhsT = KT_h slice, rhs = QT_h slice)
#     pT = exp(scoresT / 8)             (ACT, scale folded in; no max
#                                        subtraction -- scores are O(1))
#     xT[65, S]  = [V_h | 1]^T @ pT     (row 64 = softmax denominators)
#     xnorm      = xT[0:64] * (1 / xT[64])
#   outT = Wo_t^T @ xnorm_all_heads     [256, S] partial, summed on host.
import os
import numpy as np

import concourse.bass as bass
from concourse import bacc
import concourse.mybir as mybir
from concourse.tile import TileContext
from concourse.bass_utils import run_bass_kernel_spmd
from contextlib import ExitStack

B, A, S, E = 2, 6, 1024, 256
DMODEL, H, DK = 512, 8, 64
F = 256                 # per-task projection width (4 heads x 64)
OUTD = 256              # output dim (q_dim)
NT = 3                  # tasks per core
NCORES = 8
P = 128
NPAIR = B * A           # 12
CHUNK = 512             # Sq chunk (one PSUM bank of f32)

USE_F32R = os.environ.get("BASS_MM_DTYPE", "f32r") == "f32r"
ATTN_BF16 = os.environ.get("BASS_ATTN_DTYPE", "bf16") == "bf16"
PROJ_BF16 = os.environ.get("BASS_PROJ_DTYPE", "bf16") == "bf16"


def build_nc(n_tasks=NT, use_f32r=USE_F32R, attn_bf16=ATTN_BF16, proj_bf16=PROJ_BF16):
    f32 = mybir.dt.float32
    mmdt = mybir.dt.float32r if use_f32r else f32
    adt = mybir.dt.bfloat16 if attn_bf16 else mmdt
    pdt = mybir.dt.bfloat16 if proj_bf16 else mmdt
    ADD = mybir.AluOpType.add
    MULT = mybir.AluOpType.mult
    EXP = mybir.ActivationFunctionType.Exp

    nc = bacc.Bacc(None, target_bir_lowering=False, debug=False)
    qT_d = nc.declare_dram_parameter("qT", [n_tasks, E, S], pdt, isOutput=False)
    kT_d = nc.declare_dram_parameter("kT", [n_tasks, E, S], pdt, isOutput=False)
    vT_d = nc.declare_dram_parameter("vT", [n_tasks, E, S], pdt, isOutput=False)
    wq_d = nc.declare_dram_parameter("wq", [n_tasks, E, F], pdt, isOutput=False)
    wk_d = nc.declare_dram_parameter("wk", [n_tasks, E, F], pdt, isOutput=False)
    wv_d = nc.declare_dram_parameter("wv", [n_tasks, E, F], pdt, isOutput=False)
    wo_d = nc.declare_dram_parameter("wo", [n_tasks, 4, DK, OUTD], adt, isOutput=False)
    bq_d = nc.declare_dram_parameter("bq", [n_tasks, F], f32, isOutput=False)
    bk_d = nc.declare_dram_parameter("bk", [n_tasks, F], f32, isOutput=False)
    bv_d = nc.declare_dram_parameter("bv", [n_tasks, F], f32, isOutput=False)
    ones_d = nc.declare_dram_parameter("ones", [P, DK], mmdt, isOutput=False)
    out_d = nc.declare_dram_parameter("out", [n_tasks, OUTD, S], f32, isOutput=True)

    with TileContext(nc) as tc, ExitStack() as ctx:
        inbuf = ctx.enter_context(tc.tile_pool(name="inbuf", bufs=2))
        wbuf = ctx.enter_context(tc.tile_pool(name="wbuf", bufs=2))
        proj = ctx.enter_context(tc.tile_pool(name="proj", bufs=2))
        ptbuf = ctx.enter_context(tc.tile_pool(name="ptbuf", bufs=4))
        xnbuf = ctx.enter_context(tc.tile_pool(name="xnbuf", bufs=3))
        obuf = ctx.enter_context(tc.tile_pool(name="obuf", bufs=2))
        drambuf = ctx.enter_context(tc.tile_pool(name="drams", bufs=8, space="DRAM"))
        psA = ctx.enter_context(tc.tile_pool(name="psA", bufs=4, space="PSUM"))
        psX = ctx.enter_context(tc.tile_pool(name="psX", bufs=2, space="PSUM"))

        def do_proj(t):
            """Load task t's inputs and compute QT/KT/V(+ones)."""
            qT_sb = inbuf.tile([P, 2, S], pdt, tag="qT", name="qT_sb")
            kT_sb = inbuf.tile([P, 2, S], pdt, tag="kT", name="kT_sb")
            vT_sb = inbuf.tile([P, 2, S], pdt, tag="vT", name="vT_sb")
            wq_sb = wbuf.tile([P, 2, F], pdt, tag="wq", name="wq_sb")
            wk_sb = wbuf.tile([P, 2, F], pdt, tag="wk", name="wk_sb")
            wv_sb = wbuf.tile([P, 2, F], pdt, tag="wv", name="wv_sb")
            wo_sb = wbuf.tile([DK, 4, OUTD], adt, tag="wo", name="wo_sb")
            bq_sb = wbuf.tile([P, 2], f32, tag="bq", name="bq_sb")
            bk_sb = wbuf.tile([P, 2], f32, tag="bk", name="bk_sb")
            bv_bc = wbuf.tile([P, F], f32, tag="bvbc", name="bv_bc")
            # issue in first-consumer order: the first proj matmuls need wq+qT
            nc.sync.dma_start(out=wq_sb, in_=wq_d[t].rearrange("(e p) f -> p e f", p=P))
            nc.sync.dma_start(out=bq_sb, in_=bq_d[t].rearrange("(e p) -> p e", p=P))
            for ek in range(2):
                nc.sync.dma_start(
                    out=qT_sb[:, ek, :], in_=qT_d[t, 128 * ek : 128 * ek + 128, :]
                )
            nc.sync.dma_start(out=wk_sb, in_=wk_d[t].rearrange("(e p) f -> p e f", p=P))
            nc.sync.dma_start(out=bk_sb, in_=bk_d[t].rearrange("(e p) -> p e", p=P))
            for ek in range(2):
                nc.sync.dma_start(
                    out=kT_sb[:, ek, :], in_=kT_d[t, 128 * ek : 128 * ek + 128, :]
                )
            nc.sync.dma_start(out=wv_sb, in_=wv_d[t].rearrange("(e p) f -> p e f", p=P))
            nc.sync.dma_start(out=bv_bc, in_=bv_d[t].partition_broadcast(P))
            for ek in range(2):
                nc.sync.dma_start(
                    out=vT_sb[:, ek, :], in_=vT_d[t, 128 * ek : 128 * ek + 128, :]
                )
            nc.sync.dma_start(out=wo_sb, in_=wo_d[t].rearrange("h p m -> p h m"))

            qproj = proj.tile([P, 2, S], adt, tag="qproj", name="qproj")
            kproj = proj.tile([P, 2, S], adt, tag="kproj", name="kproj")
            vsb = proj.tile([P, 8, 4, DK + 1], adt, tag="vsb", name="vsb")
            nc.gpsimd.dma_start(
                out=vsb[:, :, :, DK : DK + 1],
                in_=ones_d[:, 0:32].rearrange("p (m h) -> p m h", m=8),
            )

            for dst, srct, w_sb, b_sb in (
                (qproj, qT_sb, wq_sb, bq_sb),
                (kproj, kT_sb, wk_sb, bk_sb),
            ):
                for eo in range(2):          # output F tile
                    for n in range(2):       # S chunk
                        ps = psA.tile([P, CHUNK], f32, tag="ps_small", name="ps", bufs=2)
                        for ek in range(2):  # contraction tile over E
                            nc.tensor.matmul(
                                ps,
                                lhsT=w_sb[:, ek, 128 * eo : 128 * eo + 128],
                                rhs=srct[:, ek, CHUNK * n : CHUNK * (n + 1)],
                                start=(ek == 0),
                                stop=(ek == 1),
                            )
                        nc.vector.tensor_tensor(
                            out=dst[:, eo, CHUNK * n : CHUNK * (n + 1)],
                            in0=ps,
                            in1=b_sb[:, eo : eo + 1].to_broadcast((P, CHUNK)),
                            op=ADD,
                        )

            for m in range(8):               # S tiles of V
                psv = psA.tile([P, F], f32, tag="ps_small", name="psv", bufs=2)
                for ek in range(2):
                    nc.tensor.matmul(
                        psv,
                        lhsT=vT_sb[:, ek, 128 * m : 128 * m + 128],
                        rhs=wv_sb[:, ek, :],
                        start=(ek == 0),
                        stop=(ek == 1),
                    )
                nc.vector.tensor_tensor(
                    out=vsb[:, m, :, 0:DK],
                    in0=psv.rearrange("p (h d) -> p h d", h=4),
                    in1=bv_bc.rearrange("p (h d) -> p h d", h=4),
                    op=ADD,
                )
            return qproj, kproj, vsb, wo_sb

        def do_attention(state, last=False):
            qproj, kproj, vsb, wo_sb = state
            if last:
                ones_sb = wbuf.tile([P, DK], f32, tag="ones1", name="ones_sb")
                nc.sync.dma_start(out=ones_sb, in_=ones_d[:].bitcast(f32))
            xh = xnbuf.tile([DK, 4, 2, CHUNK], adt, tag="xh", name="xh")
            for h in range(4):
                e, r0 = h // 2, 64 * (h % 2)
                psx = [psX.tile([P, CHUNK], f32, tag="psx", name="psx") for _ in range(2)]
                for m in range(8):           # Sk tiles
                    pss = [
                        psA.tile([P, CHUNK], f32, tag="pss", name="pss")
                        for _ in range(2)
                    ]
                    for n in range(2):
                        nc.tensor.matmul(
                            pss[n],
                            lhsT=kproj[r0 : r0 + 64, e, 128 * m : 128 * m + 128],
                            rhs=qproj[r0 : r0 + 64, e, CHUNK * n : CHUNK * (n + 1)],
                            start=True,
                            stop=True,
                        )
                    pt = ptbuf.tile([P, 2 * CHUNK], adt, tag="pt", name="pt")
                    for n in range(2):
                        nc.scalar.activation(
                            out=pt[:, CHUNK * n : CHUNK * (n + 1)],
                            in_=pss[n],
                            func=EXP,
                            scale=0.125,
                        )
                    for n in range(2):
                        nc.tensor.matmul(
                            psx[n][0 : DK + 1, :],
                            lhsT=vsb[:, m, h, :],
                            rhs=pt[:, CHUNK * n : CHUNK * (n + 1)],
                            start=(m == 0),
                            stop=(m == 7),
                        )
                for n in range(2):
                    px = psx[n]
                    # evacuate immediately so the AV accumulator bank frees up
                    xsb = xnbuf.tile([P, CHUNK], f32, tag="xsb", name="xsb")
                    if last and h == 3:  # tail: offload the copy to the idle ACT
                        nc.scalar.activation(
                            out=xsb[0 : DK + 1, :],
                            in_=px[0 : DK + 1, :],
                            func=mybir.ActivationFunctionType.Copy,
                        )
                    else:
                        nc.vector.tensor_copy(out=xsb[0 : DK + 1, :], in_=px[0 : DK + 1, :])
                    # reciprocal of the denominator row via 32x32 DVE transpose
                    tt = xnbuf.tile([P, CHUNK], f32, tag="tt", name="tt")
                    rec = xnbuf.tile([P, CHUNK], f32, tag="rec", name="rec")
                    tb = xnbuf.tile([P, CHUNK], f32, tag="tb", name="tb")
                    nc.vector.transpose(out=tt[DK : DK + 32, :], in_=xsb[DK : DK + 32, :])
                    with nc.allow_low_precision(reason="softmax recip"):
                        nc.vector.reciprocal(
                            out=rec[DK : DK + 32, 0:CHUNK:32],
                            in_=tt[DK : DK + 32, 0:CHUNK:32],
                        )
                    nc.vector.transpose(out=tb[DK : DK + 32, :], in_=rec[DK : DK + 32, :])
                    rb = xnbuf.tile([P, CHUNK], f32, tag="rb", name="rb")
                    if last and h == 3:
                        # tail: PE is idle here; K=1 matmul broadcast avoids the
                        # DRAM-bounce latency on the critical path
                        ps_bc = psA.tile([P, CHUNK], f32, tag="ps_small", name="ps_bc", bufs=2)
                        nc.tensor.matmul(
                            ps_bc[0:DK, :],
                            lhsT=ones_sb[DK : DK + 1, :].bitcast(f32),
                            rhs=tb[DK : DK + 1, :],
                            start=True,
                            stop=True,
                        )
                        nc.scalar.activation(
                            out=rb[0:DK, :],
                            in_=ps_bc[0:DK, :],
                            func=mybir.ActivationFunctionType.Copy,
                        )
                    else:
                        # partition-broadcast via DRAM bounce (0-stride DRAM read)
                        scr = drambuf.tile([CHUNK], f32, tag="scr", name="scr")
                        nc.sync.dma_start(out=scr, in_=tb[DK : DK + 1, :])
                        nc.gpsimd.dma_start(out=rb[0:DK, :], in_=scr.partition_broadcast(DK))
                    nc.vector.tensor_tensor(
                        out=xh[0:DK, h, n, :],
                        in0=xsb[0:DK, :],
                        in1=rb[0:DK, :],
                        op=MULT,
                    )
            return xh, wo_sb

        def do_outproj(t, xh, wo_sb):
            osb = obuf.tile([P, 2, S], f32, tag="osb", name="osb")
            for mo in range(2):
                for n in range(2):
                    pso = psA.tile([P, CHUNK], f32, tag="ps_small", name="pso", bufs=2)
                    for h in range(4):
                        nc.tensor.matmul(
                            pso,
                            lhsT=wo_sb[0:DK, h, 128 * mo : 128 * mo + 128],
                            rhs=xh[0:DK, h, n, :],
                            start=(h == 0),
                            stop=(h == 3),
                        )
                    nc.vector.tensor_copy(out=osb[:, mo, CHUNK * n : CHUNK * (n + 1)], in_=pso)
                # ship each 128-row half as soon as it is evacuated
                nc.sync.dma_start(
                    out=out_d[t, 128 * mo : 128 * mo + 128, :], in_=osb[:, mo, :]
                )

        state = do_proj(0)
        for t in range(n_tasks):
            xh, wo_sb = do_attention(state, last=(t == n_tasks - 1))
            if t + 1 < n_tasks:
                state = do_proj(t + 1)
            do_outproj(t, xh, wo_sb)

    nc.finalize()
    return nc


_cache = {}


def _get_nc():
    if "nc" not in _cache:
        _cache["nc"] = build_nc()
    return _cache["nc"]


def _tasks_of(c):
    return [NT * c + j for j in range(NT)]


def make_in_maps(query, key, value, Wq, bq, Wk, bk, Wv, bv, Wo, bo):
    import ml_dtypes

    wo_dt = ml_dtypes.bfloat16 if ATTN_BF16 else np.float32
    in_dt = ml_dtypes.bfloat16 if PROJ_BF16 else np.float32
    f = np.float32
    q = np.asarray(query, f).reshape(NPAIR, S, E)
    k = np.asarray(key, f).reshape(NPAIR, S, E)
    v = np.asarray(value, f).reshape(NPAIR, S, E)
    qT = np.ascontiguousarray(q.transpose(0, 2, 1))
    kT = np.ascontiguousarray(k.transpose(0, 2, 1))
    vT = np.ascontiguousarray(v.transpose(0, 2, 1))
    Wq_, Wk_, Wv_, Wo_ = (np.asarray(w, f) for w in (Wq, Wk, Wv, Wo))
    bq_, bk_, bv_ = (np.asarray(b, f) for b in (bq, bk, bv))

    in_maps = []
    for c in range(NCORES):
        ts = _tasks_of(c)
        pairs = [t // 2 for t in ts]
        sls = [slice(F * (t % 2), F * (t % 2) + F) for t in ts]
        in_maps.append(
            {
                "ones": np.ones((P, DK), np.float32),
                "qT": np.ascontiguousarray(qT[pairs]).astype(in_dt),
                "kT": np.ascontiguousarray(kT[pairs]).astype(in_dt),
                "vT": np.ascontiguousarray(vT[pairs]).astype(in_dt),
                "wq": np.ascontiguousarray(np.stack([Wq_[:, s] for s in sls])).astype(in_dt),
                "wk": np.ascontiguousarray(np.stack([Wk_[:, s] for s in sls])).astype(in_dt),
                "wv": np.ascontiguousarray(np.stack([Wv_[:, s] for s in sls])).astype(in_dt),
                "wo": np.ascontiguousarray(np.stack([Wo_[s, :].reshape(4, DK, OUTD) for s in sls])).astype(wo_dt),
                "bq": np.stack([bq_[s] for s in sls]),
                "bk": np.stack([bk_[s] for s in sls]),
                "bv": np.stack([bv_[s] for s in sls]),
            }
        )
    return in_maps


def assemble_output(results, bo):
    out = np.zeros((NPAIR, S, OUTD), np.float32)
    for c in range(NCORES):
        o = np.asarray(results[c]["out"])  # [NT, 256, 1024]
        for j, t in enumerate(_tasks_of(c)):
            out[t // 2] += o[j].T
    out += np.asarray(bo, np.float32)
    return out.reshape(B, A, S, OUTD)


def kernel(query, key, value, Wq, bq, Wk, bk, Wv, bv, Wo, bo):
    import time

    in_maps = make_in_maps(query, key, value, Wq, bq, Wk, bk, Wv, bv, Wo, bo)
    last_err = None
    for _ in range(3):  # the device occasionally reports a transient
        try:            # NRT_EXEC_UNIT_UNRECOVERABLE on a fresh load; retry
            res = run_bass_kernel_spmd(
                _get_nc(), in_maps, core_ids=list(range(NCORES))
            )
            out = assemble_output(res.results, bo)
            if np.isfinite(out).all():
                return out
            last_err = RuntimeError("non-finite output")
        except Exception as e:  # noqa: BLE001
            last_err = e
        time.sleep(2)
    raise last_err



# revision 2
# speedup vs baseline: 1.1695x; 1.1695x over previous
# Multi-headed attention (B=2, A=6, S=1024, E=256, d_model=512, H=8, DK=64)
# distributed over 8 NeuronCores.
#
# Decomposition: the 12 (batch, agent) pairs are each split into two
# "quad-tasks" of 4 heads (d_model halves), giving 24 tasks; each core runs
# 3 tasks (perfect balance, no duplicated FLOPs: QKV projections split
# cleanly along the head dim, the output projection's head contraction is
# summed on the host).
#
# Per-task device pipeline (no on-device transposes anywhere):
#   inputs arrive host-pre-transposed as x^T [E, S].
#   QT = Wq_t^T @ q^T   [F=256, S]      (lhsT = Wq_t, rhs = q^T)
#   KT = Wk_t^T @ k^T   [F=256, S]
#   V  = (v^T)^T @ Wv_t [S, F]          (lhsT = v^T slice, rhs = Wv_t)
#   per head h (64 rows of QT/KT):
#     scoresT[k, q] = K_h @ Q_h^T       (lhsT = KT_h slice, rhs = QT_h slice;
#                                        both 512-chunks land in one 2-bank
#                                        PSUM tile)
#     pT = exp(scoresT / 8)             (ONE [128,1024] ACT instr per k-tile;
#                                        no max subtraction -- scores O(1))
#     xT[65, S]  = [V_h | 1]^T @ pT     (row 64 = softmax denominators)
#     outT_h = Wo_h^T @ xT[0:64]        (RAW, unnormalized)
#   ship outT_h [4, 256, S] and the denominators [4, S]; the host divides
#   (normalization commutes with the per-head linear) and sums heads.
import numpy as np

import concourse.bass as bass
from concourse import bacc
import concourse.mybir as mybir
from concourse.tile import TileContext
from concourse.bass_utils import run_bass_kernel_spmd
from contextlib import ExitStack

B, A, S, E = 2, 6, 1024, 256
DMODEL, H, DK = 512, 8, 64
F = 256                 # per-task projection width (4 heads x 64)
OUTD = 256              # output dim (q_dim)
NT = 3                  # tasks per core
NCORES = 8
P = 128
NPAIR = B * A           # 12
CHUNK = 512             # Sq chunk (one PSUM bank of f32)


def build_nc(n_tasks=NT):
    f32 = mybir.dt.float32
    bf16 = mybir.dt.bfloat16
    ADD = mybir.AluOpType.add
    EXP = mybir.ActivationFunctionType.Exp

    nc = bacc.Bacc(None, target_bir_lowering=False, debug=False)
    qT_d = nc.declare_dram_parameter("qT", [n_tasks, E, S], bf16, isOutput=False)
    kT_d = nc.declare_dram_parameter("kT", [n_tasks, E, S], bf16, isOutput=False)
    vT_d = nc.declare_dram_parameter("vT", [n_tasks, E, S], bf16, isOutput=False)
    wq_d = nc.declare_dram_parameter("wq", [n_tasks, E, F], bf16, isOutput=False)
    wk_d = nc.declare_dram_parameter("wk", [n_tasks, E, F], bf16, isOutput=False)
    wv_d = nc.declare_dram_parameter("wv", [n_tasks, E, F], bf16, isOutput=False)
    wo_d = nc.declare_dram_parameter("wo", [n_tasks, 4, DK, OUTD], bf16, isOutput=False)
    bq_d = nc.declare_dram_parameter("bq", [n_tasks, F], f32, isOutput=False)
    bk_d = nc.declare_dram_parameter("bk", [n_tasks, F], f32, isOutput=False)
    bv_d = nc.declare_dram_parameter("bv", [n_tasks, F], f32, isOutput=False)
    ones_d = nc.declare_dram_parameter("ones", [P, DK], bf16, isOutput=False)
    out_d = nc.declare_dram_parameter("out", [n_tasks, 4, 2, P, S], f32, isOutput=True)
    den_d = nc.declare_dram_parameter("den", [n_tasks, 4, S], bf16, isOutput=True)

    with TileContext(nc) as tc, ExitStack() as ctx:
        inbuf = ctx.enter_context(tc.tile_pool(name="inbuf", bufs=2))
        wbuf = ctx.enter_context(tc.tile_pool(name="wbuf", bufs=2))
        proj = ctx.enter_context(tc.tile_pool(name="proj", bufs=2))
        ptbuf = ctx.enter_context(tc.tile_pool(name="ptbuf", bufs=3))
        xnbuf = ctx.enter_context(tc.tile_pool(name="xnbuf", bufs=3))
        obuf = ctx.enter_context(tc.tile_pool(name="obuf", bufs=2))
        psS = ctx.enter_context(tc.tile_pool(name="psS", bufs=2, space="PSUM"))
        psP = ctx.enter_context(tc.tile_pool(name="psP", bufs=2, space="PSUM"))
        psX = ctx.enter_context(tc.tile_pool(name="psX", bufs=2, space="PSUM"))

        def load_task(t):
            """Issue task t's input DMAs (first-consumer order)."""
            qT_sb = inbuf.tile([P, 2, S], bf16, tag="qT", name="qT_sb")
            kT_sb = inbuf.tile([P, 2, S], bf16, tag="kT", name="kT_sb")
            vT_sb = inbuf.tile([P, 2, S], bf16, tag="vT", name="vT_sb")
            wq_sb = wbuf.tile([P, 2, F], bf16, tag="wq", name="wq_sb")
            wk_sb = wbuf.tile([P, 2, F], bf16, tag="wk", name="wk_sb")
            wv_sb = wbuf.tile([P, 2, F], bf16, tag="wv", name="wv_sb")
            wo_sb = wbuf.tile([DK, 4, OUTD], bf16, tag="wo", name="wo_sb")
            bq_sb = wbuf.tile([P, 2], f32, tag="bq", name="bq_sb")
            bk_sb = wbuf.tile([P, 2], f32, tag="bk", name="bk_sb")
            bv_bc = wbuf.tile([P, F], f32, tag="bvbc", name="bv_bc")
            vsb = proj.tile([P, 8, 4, DK + 1], bf16, tag="vsb", name="vsb")
            nc.sync.dma_start(out=wq_sb, in_=wq_d[t].rearrange("(e p) f -> p e f", p=P))
            nc.sync.dma_start(out=bq_sb, in_=bq_d[t].rearrange("(e p) -> p e", p=P))
            for ek in range(2):
                nc.sync.dma_start(
                    out=qT_sb[:, ek, :], in_=qT_d[t, 128 * ek : 128 * ek + 128, :]
                )
            nc.sync.dma_start(out=wk_sb, in_=wk_d[t].rearrange("(e p) f -> p e f", p=P))
            nc.sync.dma_start(out=bk_sb, in_=bk_d[t].rearrange("(e p) -> p e", p=P))
            for ek in range(2):
                nc.sync.dma_start(
                    out=kT_sb[:, ek, :], in_=kT_d[t, 128 * ek : 128 * ek + 128, :]
                )
            nc.sync.dma_start(out=wv_sb, in_=wv_d[t].rearrange("(e p) f -> p e f", p=P))
            nc.sync.dma_start(out=bv_bc, in_=bv_d[t].partition_broadcast(P))
            for ek in range(2):
                nc.sync.dma_start(
                    out=vT_sb[:, ek, :], in_=vT_d[t, 128 * ek : 128 * ek + 128, :]
                )
            nc.sync.dma_start(out=wo_sb, in_=wo_d[t].rearrange("h p m -> p h m"))
            nc.sync.dma_start(
                out=vsb[:, :, :, DK : DK + 1],
                in_=ones_d[:, 0:32].rearrange("p (m h) -> p m h", m=8),
            )
            return qT_sb, kT_sb, vT_sb, wq_sb, wk_sb, wv_sb, wo_sb, bq_sb, bk_sb, bv_bc, vsb

        def proj_compute(ld):
            """QKV projections for a loaded task."""
            qT_sb, kT_sb, vT_sb, wq_sb, wk_sb, wv_sb, wo_sb, bq_sb, bk_sb, bv_bc, vsb = ld
            qproj = proj.tile([P, 2, S], bf16, tag="qproj", name="qproj")
            kproj = proj.tile([P, 2, S], bf16, tag="kproj", name="kproj")

            for dst, srct, w_sb, b_sb in (
                (qproj, qT_sb, wq_sb, bq_sb),
                (kproj, kT_sb, wk_sb, bk_sb),
            ):
                for eo in range(2):          # output F tile
                    ps = [
                        psP.tile([P, CHUNK], f32, tag="psp", name="ps")
                        for _ in range(2)
                    ]
                    for ek in range(2):      # contraction tile over E
                        for n in range(2):   # S chunk (reuses the weights)
                            nc.tensor.matmul(
                                ps[n],
                                lhsT=w_sb[:, ek, 128 * eo : 128 * eo + 128],
                                rhs=srct[:, ek, CHUNK * n : CHUNK * (n + 1)],
                                start=(ek == 0),
                                stop=(ek == 1),
                            )
                    for n in range(2):
                        nc.vector.tensor_tensor(
                            out=dst[:, eo, CHUNK * n : CHUNK * (n + 1)],
                            in0=ps[n],
                            in1=b_sb[:, eo : eo + 1].to_broadcast((P, CHUNK)),
                            op=ADD,
                        )

            for m in range(8):               # S tiles of V
                psv = psP.tile([P, F], f32, tag="psp", name="psv")
                for ek in range(2):
                    nc.tensor.matmul(
                        psv,
                        lhsT=vT_sb[:, ek, 128 * m : 128 * m + 128],
                        rhs=wv_sb[:, ek, :],
                        start=(ek == 0),
                        stop=(ek == 1),
                    )
                nc.vector.tensor_tensor(
                    out=vsb[:, m, :, 0:DK],
                    in0=psv.rearrange("p (h d) -> p h d", h=4),
                    in1=bv_bc.rearrange("p (h d) -> p h d", h=4),
                    op=ADD,
                )
            return qproj, kproj, vsb, wo_sb

        def outproj_block(t, h, xsb, wo_sb):
            """RAW per-head output projection + ship to DRAM."""
            osb = obuf.tile([P, 2, 2, CHUNK], f32, tag="osb", name="osb")
            for mo in range(2):
                pso = [
                    psP.tile([P, CHUNK], f32, tag="psp", name="pso")
                    for _ in range(2)
                ]
                for n in range(2):
                    nc.tensor.matmul(
                        pso[n],
                        lhsT=wo_sb[0:DK, h, 128 * mo : 128 * mo + 128],
                        rhs=xsb[0:DK, n, :],
                        start=True,
                        stop=True,
                    )
                for n in range(2):
                    nc.vector.tensor_copy(out=osb[:, mo, n, :], in_=pso[n])
                nc.gpsimd.dma_start(out=out_d[t, h, mo], in_=osb[:, mo])

        state = proj_compute(load_task(0))
        pending = None                       # (t, h, xsb, wo_sb) awaiting outproj
        for t in range(n_tasks):
            qproj, kproj, vsb, wo_sb = state
            ld_next = None
            for h in range(4):
                e, r0 = h // 2, 64 * (h % 2)
                psx = [
                    psX.tile([P, CHUNK], f32, tag="psx", name="psx")
                    for _ in range(2)
                ]
                for m in range(8):           # Sk tiles
                    pss = psS.tile([P, 2 * CHUNK], f32, tag="pss", name="pss")
                    for n in range(2):
                        nc.tensor.matmul(
                            pss[:, CHUNK * n : CHUNK * (n + 1)],
                            lhsT=kproj[r0 : r0 + 64, e, 128 * m : 128 * m + 128],
                            rhs=qproj[r0 : r0 + 64, e, CHUNK * n : CHUNK * (n + 1)],
                            start=True,
                            stop=True,
                        )
                    pt = ptbuf.tile([P, 2 * CHUNK], bf16, tag="pt", name="pt")
                    nc.scalar.activation(out=pt, in_=pss, func=EXP, scale=0.125)
                    if m == 1:
                        # deferred work rides the exp latency window
                        if pending is not None:
                            outproj_block(*pending)
                            pending = None
                        if h == 0 and t + 1 < n_tasks:
                            ld_next = load_task(t + 1)
                        if h == 3 and t + 1 < n_tasks:
                            next_state = proj_compute(ld_next)
                    for n in range(2):
                        nc.tensor.matmul(
                            psx[n][0 : DK + 1, :],
                            lhsT=vsb[:, m, h, :],
                            rhs=pt[:, CHUNK * n : CHUNK * (n + 1)],
                            start=(m == 0),
                            stop=(m == 7),
                        )
                xsb = xnbuf.tile([P, 2, CHUNK], bf16, tag="xsb", name="xsb")
                for n in range(2):
                    nc.vector.tensor_copy(
                        out=xsb[0 : DK + 1, n, :], in_=psx[n][0 : DK + 1, :]
                    )
                nc.gpsimd.dma_start(out=den_d[t, h], in_=xsb[DK : DK + 1, :, :])
                pending = (t, h, xsb, wo_sb)
            if t + 1 < n_tasks:
                state = next_state
        outproj_block(*pending)

    nc.finalize()
    return nc


_cache = {}


def _get_nc():
    if "nc" not in _cache:
        _cache["nc"] = build_nc()
    return _cache["nc"]


def _tasks_of(c):
    return [NT * c + j for j in range(NT)]


def make_in_maps(query, key, value, Wq, bq, Wk, bk, Wv, bv, Wo, bo):
    import ml_dtypes

    in_dt = ml_dtypes.bfloat16
    f = np.float32
    q = np.asarray(query, f).reshape(NPAIR, S, E)
    k = np.asarray(key, f).reshape(NPAIR, S, E)
    v = np.asarray(value, f).reshape(NPAIR, S, E)
    qT = np.ascontiguousarray(q.transpose(0, 2, 1))
    kT = np.ascontiguousarray(k.transpose(0, 2, 1))
    vT = np.ascontiguousarray(v.transpose(0, 2, 1))
    Wq_, Wk_, Wv_, Wo_ = (np.asarray(w, f) for w in (Wq, Wk, Wv, Wo))
    bq_, bk_, bv_ = (np.asarray(b, f) for b in (bq, bk, bv))

    in_maps = []
    for c in range(NCORES):
        ts = _tasks_of(c)
        pairs = [t // 2 for t in ts]
        sls = [slice(F * (t % 2), F * (t % 2) + F) for t in ts]
        in_maps.append(
            {
                "ones": np.ones((P, DK), ml_dtypes.bfloat16),
                "qT": np.ascontiguousarray(qT[pairs]).astype(in_dt),
                "kT": np.ascontiguousarray(kT[pairs]).astype(in_dt),
                "vT": np.ascontiguousarray(vT[pairs]).astype(in_dt),
                "wq": np.ascontiguousarray(np.stack([Wq_[:, s] for s in sls])).astype(in_dt),
                "wk": np.ascontiguousarray(np.stack([Wk_[:, s] for s in sls])).astype(in_dt),
                "wv": np.ascontiguousarray(np.stack([Wv_[:, s] for s in sls])).astype(in_dt),
                "wo": np.ascontiguousarray(np.stack([Wo_[s, :].reshape(4, DK, OUTD) for s in sls])).astype(in_dt),
                "bq": np.stack([bq_[s] for s in sls]),
                "bk": np.stack([bk_[s] for s in sls]),
                "bv": np.stack([bv_[s] for s in sls]),
            }
        )
    return in_maps


def assemble_output(results, bo):
    out = np.zeros((NPAIR, S, OUTD), np.float32)
    for c in range(NCORES):
        o = np.asarray(results[c]["out"])                 # [NT, 4, 2, 128, S]
        den = np.asarray(results[c]["den"], np.float32)   # [NT, 4, S]
        for j, t in enumerate(_tasks_of(c)):
            x = o[j].reshape(4, OUTD, S) / den[j][:, None, :]
            out[t // 2] += x.sum(0).T
    out += np.asarray(bo, np.float32)
    return out.reshape(B, A, S, OUTD)


def kernel(query, key, value, Wq, bq, Wk, bk, Wv, bv, Wo, bo):
    import time

    in_maps = make_in_maps(query, key, value, Wq, bq, Wk, bk, Wv, bv, Wo, bo)
    last_err = None
    for _ in range(3):  # the device occasionally reports a transient
        try:            # NRT_EXEC_UNIT_UNRECOVERABLE on a fresh load; retry
            res = run_bass_kernel_spmd(
                _get_nc(), in_maps, core_ids=list(range(NCORES))
            )
            out = assemble_output(res.results, bo)
            if np.isfinite(out).all():
                return out
            last_err = RuntimeError("non-finite output")
        except Exception as e:  # noqa: BLE001
            last_err = e
        time.sleep(2)
    raise last_err


# revision 8
# speedup vs baseline: 1.3183x; 1.1272x over previous
# Multi-headed attention (B=2, A=6, S=1024, E=256, d_model=512, H=8, DK=64)
# distributed over 8 NeuronCores.
#
# Decomposition: the 12 (batch, agent) pairs are each split into two
# "quad-tasks" of 4 heads (d_model halves), giving 24 tasks; each core runs
# 3 tasks (perfect balance, no duplicated FLOPs: QKV projections split
# cleanly along the head dim, the output projection's head contraction is
# summed on the host).
#
# Per-task device pipeline (no on-device transposes anywhere):
#   inputs arrive host-pre-transposed as x^T [E, S].
#   QT = Wq_t^T @ q^T   [F=256, S]      (lhsT = Wq_t, rhs = q^T)
#   KT = Wk_t^T @ k^T   [F=256, S]
#   V  = (v^T)^T @ Wv_t [S, F]          (lhsT = v^T slice, rhs = Wv_t)
#   per head h (64 rows of QT/KT):
#     scoresT[k, q] = K_h @ Q_h^T       (lhsT = KT_h slice, rhs = QT_h slice;
#                                        both 512-chunks land in one 2-bank
#                                        PSUM tile)
#     pT = exp(scoresT / 8)             (ONE [128,1024] ACT instr per k-tile;
#                                        no max subtraction -- scores O(1))
#     xT[65, S]  = [V_h | 1]^T @ pT     (row 64 = softmax denominators)
#     outT_h = Wo_h^T @ xT[0:64]        (RAW, unnormalized)
#   ship outT_h [4, 256, S] and the denominators [4, S]; the host divides
#   (normalization commutes with the per-head linear) and sums heads.
import numpy as np

import concourse.bass as bass
from concourse import bacc
import concourse.mybir as mybir
from concourse.tile import TileContext
from concourse.bass_utils import run_bass_kernel_spmd
from contextlib import ExitStack

B, A, S, E = 2, 6, 1024, 256
DMODEL, H, DK = 512, 8, 64
F = 256                 # per-task projection width (4 heads x 64)
OUTD = 256              # output dim (q_dim)
NT = 3                  # tasks per core
NCORES = 8
P = 128
NPAIR = B * A           # 12
CHUNK = 512             # Sq chunk (one PSUM bank of f32)


def build_nc(n_tasks=NT):
    f32 = mybir.dt.float32
    bf16 = mybir.dt.bfloat16
    ADD = mybir.AluOpType.add
    EXP = mybir.ActivationFunctionType.Exp

    nc = bacc.Bacc(None, target_bir_lowering=False, debug=False)
    qT_d = nc.declare_dram_parameter("qT", [n_tasks, E, S], bf16, isOutput=False)
    kT_d = nc.declare_dram_parameter("kT", [n_tasks, E, S], bf16, isOutput=False)
    vT_d = nc.declare_dram_parameter("vT", [n_tasks, E, S], bf16, isOutput=False)
    wq_d = nc.declare_dram_parameter("wq", [n_tasks, E, F], bf16, isOutput=False)
    wk_d = nc.declare_dram_parameter("wk", [n_tasks, E, F], bf16, isOutput=False)
    wv_d = nc.declare_dram_parameter("wv", [n_tasks, E, F], bf16, isOutput=False)
    wo_d = nc.declare_dram_parameter("wo", [n_tasks, 4, DK, OUTD], bf16, isOutput=False)
    bq_d = nc.declare_dram_parameter("bq", [n_tasks, F], f32, isOutput=False)
    bk_d = nc.declare_dram_parameter("bk", [n_tasks, F], f32, isOutput=False)
    bv_d = nc.declare_dram_parameter("bv", [n_tasks, F], f32, isOutput=False)
    ones_d = nc.declare_dram_parameter("ones", [P, DK], bf16, isOutput=False)
    out_d = nc.declare_dram_parameter("out", [n_tasks, 4, 2, P, S], bf16, isOutput=True)
    den_d = nc.declare_dram_parameter("den", [n_tasks, 4, S], bf16, isOutput=True)

    with TileContext(nc) as tc, ExitStack() as ctx:
        inbuf = ctx.enter_context(tc.tile_pool(name="inbuf", bufs=2))
        wbuf = ctx.enter_context(tc.tile_pool(name="wbuf", bufs=2))
        proj = ctx.enter_context(tc.tile_pool(name="proj", bufs=2))
        ptbuf = ctx.enter_context(tc.tile_pool(name="ptbuf", bufs=3))
        xnbuf = ctx.enter_context(tc.tile_pool(name="xnbuf", bufs=3))
        obuf = ctx.enter_context(tc.tile_pool(name="obuf", bufs=2))
        psS = ctx.enter_context(tc.tile_pool(name="psS", bufs=2, space="PSUM"))
        psP = ctx.enter_context(tc.tile_pool(name="psP", bufs=2, space="PSUM"))
        psX = ctx.enter_context(tc.tile_pool(name="psX", bufs=2, space="PSUM"))

        def load_task(t):
            """Issue task t's input DMAs (first-consumer order)."""
            qT_sb = inbuf.tile([P, 2, S], bf16, tag="qT", name="qT_sb")
            kT_sb = inbuf.tile([P, 2, S], bf16, tag="kT", name="kT_sb")
            vT_sb = inbuf.tile([P, 2, S], bf16, tag="vT", name="vT_sb")
            wq_sb = wbuf.tile([P, 2, F], bf16, tag="wq", name="wq_sb")
            wk_sb = wbuf.tile([P, 2, F], bf16, tag="wk", name="wk_sb")
            wv_sb = wbuf.tile([P, 2, F], bf16, tag="wv", name="wv_sb")
            wo_sb = wbuf.tile([DK, 4, OUTD], bf16, tag="wo", name="wo_sb")
            bq_sb = wbuf.tile([P, 2], f32, tag="bq", name="bq_sb")
            bk_sb = wbuf.tile([P, 2], f32, tag="bk", name="bk_sb")
            bv_bc = wbuf.tile([P, F], f32, tag="bvbc", name="bv_bc")
            vsb = proj.tile([P, 8, 4, DK + 1], bf16, tag="vsb", name="vsb")
            nc.sync.dma_start(out=wq_sb, in_=wq_d[t].rearrange("(e p) f -> p e f", p=P))
            for ek in range(2):
                nc.sync.dma_start(
                    out=qT_sb[:, ek, :], in_=qT_d[t, 128 * ek : 128 * ek + 128, :]
                )
            nc.sync.dma_start(out=wk_sb, in_=wk_d[t].rearrange("(e p) f -> p e f", p=P))
            for ek in range(2):
                nc.sync.dma_start(
                    out=kT_sb[:, ek, :], in_=kT_d[t, 128 * ek : 128 * ek + 128, :]
                )
            nc.sync.dma_start(out=bq_sb, in_=bq_d[t].rearrange("(e p) -> p e", p=P))
            nc.sync.dma_start(out=bk_sb, in_=bk_d[t].rearrange("(e p) -> p e", p=P))
            nc.sync.dma_start(out=wv_sb, in_=wv_d[t].rearrange("(e p) f -> p e f", p=P))
            for ek in range(2):
                nc.sync.dma_start(
                    out=vT_sb[:, ek, :], in_=vT_d[t, 128 * ek : 128 * ek + 128, :]
                )
            nc.sync.dma_start(out=bv_bc, in_=bv_d[t].partition_broadcast(P))
            nc.sync.dma_start(out=wo_sb, in_=wo_d[t].rearrange("h p m -> p h m"))
            nc.sync.dma_start(
                out=vsb[:, :, :, DK : DK + 1],
                in_=ones_d[:, 0:32].rearrange("p (m h) -> p m h", m=8),
            )
            return qT_sb, kT_sb, vT_sb, wq_sb, wk_sb, wv_sb, wo_sb, bq_sb, bk_sb, bv_bc, vsb

        def qk_proj(ld, dsts, eo):
            """Q and K projections for one F tile (eo)."""
            qT_sb, kT_sb, vT_sb, wq_sb, wk_sb, wv_sb, wo_sb, bq_sb, bk_sb, bv_bc, vsb = ld
            qproj, kproj = dsts
            for dst, srct, w_sb, b_sb in (
                (qproj, qT_sb, wq_sb, bq_sb),
                (kproj, kT_sb, wk_sb, bk_sb),
            ):
                ps = [
                    psP.tile([P, CHUNK], f32, tag="psp", name="ps")
                    for _ in range(2)
                ]
                for ek in range(2):          # contraction tile over E
                    for n in range(2):       # S chunk (reuses the weights)
                        nc.tensor.matmul(
                            ps[n],
                            lhsT=w_sb[:, ek, 128 * eo : 128 * eo + 128],
                            rhs=srct[:, ek, CHUNK * n : CHUNK * (n + 1)],
                            start=(ek == 0),
                            stop=(ek == 1),
                        )
                for n in range(2):
                    nc.vector.tensor_tensor(
                        out=dst[:, eo, CHUNK * n : CHUNK * (n + 1)],
                        in0=ps[n],
                        in1=b_sb[:, eo : eo + 1].to_broadcast((P, CHUNK)),
                        op=ADD,
                    )

        def v_proj(ld, m):
            """V projection for one S tile (m)."""
            qT_sb, kT_sb, vT_sb, wq_sb, wk_sb, wv_sb, wo_sb, bq_sb, bk_sb, bv_bc, vsb = ld
            psv = psP.tile([P, F], f32, tag="psp", name="psv")
            for ek in range(2):
                nc.tensor.matmul(
                    psv,
                    lhsT=vT_sb[:, ek, 128 * m : 128 * m + 128],
                    rhs=wv_sb[:, ek, :],
                    start=(ek == 0),
                    stop=(ek == 1),
                )
            nc.vector.tensor_tensor(
                out=vsb[:, m, :, 0:DK],
                in0=psv.rearrange("p (h d) -> p h d", h=4),
                in1=bv_bc.rearrange("p (h d) -> p h d", h=4),
                op=ADD,
            )

        def proj_compute(ld):
            """QKV projections for a loaded task (compact form)."""
            qproj = proj.tile([P, 2, S], bf16, tag="qproj", name="qproj")
            kproj = proj.tile([P, 2, S], bf16, tag="kproj", name="kproj")
            for eo in range(2):
                qk_proj(ld, (qproj, kproj), eo)
            for m in range(8):
                v_proj(ld, m)
            return qproj, kproj, ld[10], ld[6]

        def outproj_block(t, h, xsb, wo_sb):
            """RAW per-head output projection + ship to DRAM."""
            osb = obuf.tile([P, 2, 2, CHUNK], bf16, tag="osb", name="osb")
            for mo in range(2):
                pso = [
                    psP.tile([P, CHUNK], f32, tag="psp", name="pso")
                    for _ in range(2)
                ]
                for n in range(2):
                    nc.tensor.matmul(
                        pso[n],
                        lhsT=wo_sb[0:DK, h, 128 * mo : 128 * mo + 128],
                        rhs=xsb[0:DK, n, :],
                        start=True,
                        stop=True,
                    )
                for n in range(2):
                    nc.vector.tensor_copy(out=osb[:, mo, n, :], in_=pso[n])
                nc.gpsimd.dma_start(out=out_d[t, h, mo], in_=osb[:, mo])

        # Warm the PE p-state during the initial input-DMA wait: small
        # matmuls on a memset tile ramp the clock before real work lands.
        warm = wbuf.tile([P, P], bf16, tag="warm", name="warm")
        nc.gpsimd.memset(warm, 0.0)
        for w in range(24):
            psw = psP.tile([P, 64], f32, tag="psp", name="psw")
            nc.tensor.matmul(psw, lhsT=warm, rhs=warm[:, 0:64], start=True, stop=True)

        ld = load_task(0)
        qproj0 = proj.tile([P, 2, S], bf16, tag="qproj", name="qproj")
        kproj0 = proj.tile([P, 2, S], bf16, tag="kproj", name="kproj")
        qk_proj(ld, (qproj0, kproj0), 0)   # eo=1 deferred into the h0/h1 loop
        v_proj(ld, 0)
        v_proj(ld, 1)
        state = (qproj0, kproj0, ld[10], ld[6])
        pending = None                       # (t, h, xsb, wo_sb) awaiting outproj
        for t in range(n_tasks):
            qproj, kproj, vsb, wo_sb = state
            ld_next = None
            for h in range(4):
                e, r0 = h // 2, 64 * (h % 2)
                psx = [
                    psX.tile([P, CHUNK], f32, tag="psx", name="psx")
                    for _ in range(2)
                ]
                for m in range(8):           # Sk tiles
                    pss = psS.tile([P, 2 * CHUNK], f32, tag="pss", name="pss")
                    for n in range(2):
                        nc.tensor.matmul(
                            pss[:, CHUNK * n : CHUNK * (n + 1)],
                            lhsT=kproj[r0 : r0 + 64, e, 128 * m : 128 * m + 128],
                            rhs=qproj[r0 : r0 + 64, e, CHUNK * n : CHUNK * (n + 1)],
                            start=True,
                            stop=True,
                        )
                    pt = ptbuf.tile([P, 2 * CHUNK], bf16, tag="pt", name="pt")
                    nc.scalar.activation(out=pt, in_=pss, func=EXP, scale=0.125)
                    if t == 0 and h == 0 and m < 6:
                        v_proj(ld, m + 2)    # ride the exp latency window
                    if t == 0 and h == 1 and m == 1:
                        qk_proj(ld, (qproj, kproj), 1)
                    if m == 1:
                        # deferred work rides the exp latency window
                        if pending is not None:
                            outproj_block(*pending)
                            pending = None
                        if h == 0 and t + 1 < n_tasks:
                            ld_next = load_task(t + 1)
                        if h == 3 and t + 1 < n_tasks:
                            next_state = proj_compute(ld_next)
                    for n in range(2):
                        nc.tensor.matmul(
                            psx[n][0 : DK + 1, :],
                            lhsT=vsb[:, m, h, :],
                            rhs=pt[:, CHUNK * n : CHUNK * (n + 1)],
                            start=(m == 0),
                            stop=(m == 7),
                        )
                xsb = xnbuf.tile([P, 2, CHUNK], bf16, tag="xsb", name="xsb")
                # split the evacuation across ACT and DVE so the PSUM banks
                # free before the next head's first AV matmul needs them
                nc.scalar.activation(
                    out=xsb[0 : DK + 1, 0, :],
                    in_=psx[0][0 : DK + 1, :],
                    func=mybir.ActivationFunctionType.Copy,
                )
                nc.vector.tensor_copy(
                    out=xsb[0 : DK + 1, 1, :], in_=psx[1][0 : DK + 1, :]
                )
                nc.gpsimd.dma_start(out=den_d[t, h], in_=xsb[DK : DK + 1, :, :])
                pending = (t, h, xsb, wo_sb)
            if t + 1 < n_tasks:
                state = next_state
        outproj_block(*pending)

    nc.finalize()
    return nc


_cache = {}


def _get_nc():
    if "nc" not in _cache:
        _cache["nc"] = build_nc()
    return _cache["nc"]


def _tasks_of(c):
    return [NT * c + j for j in range(NT)]


def make_in_maps(query, key, value, Wq, bq, Wk, bk, Wv, bv, Wo, bo):
    import ml_dtypes

    in_dt = ml_dtypes.bfloat16
    f = np.float32
    q = np.asarray(query, f).reshape(NPAIR, S, E)
    k = np.asarray(key, f).reshape(NPAIR, S, E)
    v = np.asarray(value, f).reshape(NPAIR, S, E)
    qT = np.ascontiguousarray(q.transpose(0, 2, 1))
    kT = np.ascontiguousarray(k.transpose(0, 2, 1))
    vT = np.ascontiguousarray(v.transpose(0, 2, 1))
    Wq_, Wk_, Wv_, Wo_ = (np.asarray(w, f) for w in (Wq, Wk, Wv, Wo))
    bq_, bk_, bv_ = (np.asarray(b, f) for b in (bq, bk, bv))

    in_maps = []
    for c in range(NCORES):
        ts = _tasks_of(c)
        pairs = [t // 2 for t in ts]
        sls = [slice(F * (t % 2), F * (t % 2) + F) for t in ts]
        in_maps.append(
            {
                "ones": np.ones((P, DK), ml_dtypes.bfloat16),
                "qT": np.ascontiguousarray(qT[pairs]).astype(in_dt),
                "kT": np.ascontiguousarray(kT[pairs]).astype(in_dt),
                "vT": np.ascontiguousarray(vT[pairs]).astype(in_dt),
                "wq": np.ascontiguousarray(np.stack([Wq_[:, s] for s in sls])).astype(in_dt),
                "wk": np.ascontiguousarray(np.stack([Wk_[:, s] for s in sls])).astype(in_dt),
                "wv": np.ascontiguousarray(np.stack([Wv_[:, s] for s in sls])).astype(in_dt),
                "wo": np.ascontiguousarray(np.stack([Wo_[s, :].reshape(4, DK, OUTD) for s in sls])).astype(in_dt),
                "bq": np.stack([bq_[s] for s in sls]),
                "bk": np.stack([bk_[s] for s in sls]),
                "bv": np.stack([bv_[s] for s in sls]),
            }
        )
    return in_maps


def assemble_output(results, bo):
    out = np.zeros((NPAIR, S, OUTD), np.float32)
    for c in range(NCORES):
        o = np.asarray(results[c]["out"], np.float32)     # [NT, 4, 2, 128, S]
        den = np.asarray(results[c]["den"], np.float32)   # [NT, 4, S]
        for j, t in enumerate(_tasks_of(c)):
            x = o[j].reshape(4, OUTD, S) / den[j][:, None, :]
            out[t // 2] += x.sum(0).T
    out += np.asarray(bo, np.float32)
    return out.reshape(B, A, S, OUTD)


def kernel(query, key, value, Wq, bq, Wk, bk, Wv, bv, Wo, bo):
    import time

    in_maps = make_in_maps(query, key, value, Wq, bq, Wk, bk, Wv, bv, Wo, bo)
    last_err = None
    for _ in range(3):  # the device occasionally reports a transient
        try:            # NRT_EXEC_UNIT_UNRECOVERABLE on a fresh load; retry
            res = run_bass_kernel_spmd(
                _get_nc(), in_maps, core_ids=list(range(NCORES))
            )
            out = assemble_output(res.results, bo)
            if np.isfinite(out).all():
                return out
            last_err = RuntimeError("non-finite output")
        except Exception as e:  # noqa: BLE001
            last_err = e
        time.sleep(2)
    raise last_err


# revision 13
# speedup vs baseline: 1.3372x; 1.0143x over previous
# Multi-headed attention (B=2, A=6, S=1024, E=256, d_model=512, H=8, DK=64)
# distributed over 8 NeuronCores.
#
# Decomposition: the 12 (batch, agent) pairs are each split into two
# "quad-tasks" of 4 heads (d_model halves), giving 24 tasks; each core runs
# 3 tasks (perfect balance, no duplicated FLOPs: QKV projections split
# cleanly along the head dim, the output projection's head contraction is
# summed on the host).
#
# Per-task device pipeline (no on-device transposes anywhere):
#   inputs arrive host-pre-transposed as x^T [E, S].
#   QT = Wq_t^T @ q^T   [F=256, S]      (lhsT = Wq_t, rhs = q^T)
#   KT = Wk_t^T @ k^T   [F=256, S]
#   V  = (v^T)^T @ Wv_t [S, F]          (lhsT = v^T slice, rhs = Wv_t)
#   per head h (64 rows of QT/KT):
#     scoresT[k, q] = K_h @ Q_h^T       (lhsT = KT_h slice, rhs = QT_h slice;
#                                        both 512-chunks land in one 2-bank
#                                        PSUM tile)
#     pT = exp(scoresT / 8)             (ONE [128,1024] ACT instr per k-tile;
#                                        no max subtraction -- scores O(1))
#     xT[65, S]  = [V_h | 1]^T @ pT     (row 64 = softmax denominators)
#     outT_h = Wo_h^T @ xT[0:64]        (RAW, unnormalized)
#   ship outT_h [4, 256, S] and the denominators [4, S]; the host divides
#   (normalization commutes with the per-head linear) and sums heads.
import numpy as np

import concourse.bass as bass
from concourse import bacc
import concourse.mybir as mybir
from concourse.tile import TileContext
from concourse.bass_utils import run_bass_kernel_spmd
from contextlib import ExitStack

B, A, S, E = 2, 6, 1024, 256
DMODEL, H, DK = 512, 8, 64
F = 256                 # per-task projection width (4 heads x 64)
OUTD = 256              # output dim (q_dim)
NT = 3                  # tasks per core
NCORES = 8
P = 128
NPAIR = B * A           # 12
CHUNK = 512             # Sq chunk (one PSUM bank of f32)


def build_nc(n_tasks=NT):
    f32 = mybir.dt.float32
    bf16 = mybir.dt.bfloat16
    ADD = mybir.AluOpType.add
    EXP = mybir.ActivationFunctionType.Exp

    nc = bacc.Bacc(None, target_bir_lowering=False, debug=False)
    qT_d = nc.declare_dram_parameter("qT", [n_tasks, E, S], bf16, isOutput=False)
    kT_d = nc.declare_dram_parameter("kT", [n_tasks, E, S], bf16, isOutput=False)
    vT_d = nc.declare_dram_parameter("vT", [n_tasks, E, S], bf16, isOutput=False)
    wq_d = nc.declare_dram_parameter("wq", [n_tasks, E, F], bf16, isOutput=False)
    wk_d = nc.declare_dram_parameter("wk", [n_tasks, E, F], bf16, isOutput=False)
    wv_d = nc.declare_dram_parameter("wv", [n_tasks, E, F], bf16, isOutput=False)
    wo_d = nc.declare_dram_parameter("wo", [n_tasks, 4, DK, OUTD], bf16, isOutput=False)
    bq_d = nc.declare_dram_parameter("bq", [n_tasks, F], f32, isOutput=False)
    bk_d = nc.declare_dram_parameter("bk", [n_tasks, F], f32, isOutput=False)
    bv_d = nc.declare_dram_parameter("bv", [n_tasks, F], f32, isOutput=False)
    ones_d = nc.declare_dram_parameter("ones", [P, DK], bf16, isOutput=False)
    out_d = nc.declare_dram_parameter("out", [n_tasks, 4, 2, P, S], bf16, isOutput=True)
    den_d = nc.declare_dram_parameter("den", [n_tasks, 4, S], bf16, isOutput=True)

    with TileContext(nc) as tc, ExitStack() as ctx:
        inbuf = ctx.enter_context(tc.tile_pool(name="inbuf", bufs=2))
        wbuf = ctx.enter_context(tc.tile_pool(name="wbuf", bufs=2))
        proj = ctx.enter_context(tc.tile_pool(name="proj", bufs=2))
        ptbuf = ctx.enter_context(tc.tile_pool(name="ptbuf", bufs=3))
        xnbuf = ctx.enter_context(tc.tile_pool(name="xnbuf", bufs=3))
        obuf = ctx.enter_context(tc.tile_pool(name="obuf", bufs=2))
        psS = ctx.enter_context(tc.tile_pool(name="psS", bufs=2, space="PSUM"))
        psP = ctx.enter_context(tc.tile_pool(name="psP", bufs=2, space="PSUM"))
        psX = ctx.enter_context(tc.tile_pool(name="psX", bufs=2, space="PSUM"))

        def load_task(t, spread=False):
            """Issue task t's input DMAs (first-consumer order). With
            spread=True (cold start) the loads fan out over four engine
            queues so the transfers land in parallel."""
            qT_sb = inbuf.tile([P, 2, S], bf16, tag="qT", name="qT_sb")
            kT_sb = inbuf.tile([P, 2, S], bf16, tag="kT", name="kT_sb")
            vT_sb = inbuf.tile([P, 2, S], bf16, tag="vT", name="vT_sb")
            wq_sb = wbuf.tile([P, 2, F], bf16, tag="wq", name="wq_sb")
            wk_sb = wbuf.tile([P, 2, F], bf16, tag="wk", name="wk_sb")
            wv_sb = wbuf.tile([P, 2, F], bf16, tag="wv", name="wv_sb")
            wo_sb = wbuf.tile([DK, 4, OUTD], bf16, tag="wo", name="wo_sb")
            bq_sb = wbuf.tile([P, 2], f32, tag="bq", name="bq_sb")
            bk_sb = wbuf.tile([P, 2], f32, tag="bk", name="bk_sb")
            bv_bc = wbuf.tile([P, F], f32, tag="bvbc", name="bv_bc")
            vsb = proj.tile([P, 8, 4, DK + 1], bf16, tag="vsb", name="vsb")
            if spread:
                qq, qk, qv, qb = nc.sync, nc.scalar, nc.gpsimd, nc.sync
            else:
                qq = qk = qv = qb = nc.sync
            qq.dma_start(out=wq_sb, in_=wq_d[t].rearrange("(e p) f -> p e f", p=P))
            for ek in range(2):
                qq.dma_start(
                    out=qT_sb[:, ek, :], in_=qT_d[t, 128 * ek : 128 * ek + 128, :]
                )
            qk.dma_start(out=wk_sb, in_=wk_d[t].rearrange("(e p) f -> p e f", p=P))
            for ek in range(2):
                qk.dma_start(
                    out=kT_sb[:, ek, :], in_=kT_d[t, 128 * ek : 128 * ek + 128, :]
                )
            qb.dma_start(out=bq_sb, in_=bq_d[t].rearrange("(e p) -> p e", p=P))
            qb.dma_start(out=bk_sb, in_=bk_d[t].rearrange("(e p) -> p e", p=P))
            qv.dma_start(out=wv_sb, in_=wv_d[t].rearrange("(e p) f -> p e f", p=P))
            for ek in range(2):
                qv.dma_start(
                    out=vT_sb[:, ek, :], in_=vT_d[t, 128 * ek : 128 * ek + 128, :]
                )
            qb.dma_start(out=bv_bc, in_=bv_d[t].partition_broadcast(P))
            qb.dma_start(out=wo_sb, in_=wo_d[t].rearrange("h p m -> p h m"))
            qv.dma_start(
                out=vsb[:, :, :, DK : DK + 1],
                in_=ones_d[:, 0:32].rearrange("p (m h) -> p m h", m=8),
            )
            return qT_sb, kT_sb, vT_sb, wq_sb, wk_sb, wv_sb, wo_sb, bq_sb, bk_sb, bv_bc, vsb

        def qk_proj(ld, dsts, eo):
            """Q and K projections for one F tile (eo)."""
            qT_sb, kT_sb, vT_sb, wq_sb, wk_sb, wv_sb, wo_sb, bq_sb, bk_sb, bv_bc, vsb = ld
            qproj, kproj = dsts
            for dst, srct, w_sb, b_sb in (
                (qproj, qT_sb, wq_sb, bq_sb),
                (kproj, kT_sb, wk_sb, bk_sb),
            ):
                ps = [
                    psP.tile([P, CHUNK], f32, tag="psp", name="ps")
                    for _ in range(2)
                ]
                for ek in range(2):          # contraction tile over E
                    for n in range(2):       # S chunk (reuses the weights)
                        nc.tensor.matmul(
                            ps[n],
                            lhsT=w_sb[:, ek, 128 * eo : 128 * eo + 128],
                            rhs=srct[:, ek, CHUNK * n : CHUNK * (n + 1)],
                            start=(ek == 0),
                            stop=(ek == 1),
                        )
                for n in range(2):
                    nc.vector.tensor_tensor(
                        out=dst[:, eo, CHUNK * n : CHUNK * (n + 1)],
                        in0=ps[n],
                        in1=b_sb[:, eo : eo + 1].to_broadcast((P, CHUNK)),
                        op=ADD,
                    )

        def v_proj(ld, m):
            """V projection for one S tile (m)."""
            qT_sb, kT_sb, vT_sb, wq_sb, wk_sb, wv_sb, wo_sb, bq_sb, bk_sb, bv_bc, vsb = ld
            psv = psP.tile([P, F], f32, tag="psp", name="psv")
            for ek in range(2):
                nc.tensor.matmul(
                    psv,
                    lhsT=vT_sb[:, ek, 128 * m : 128 * m + 128],
                    rhs=wv_sb[:, ek, :],
                    start=(ek == 0),
                    stop=(ek == 1),
                )
            nc.vector.tensor_tensor(
                out=vsb[:, m, :, 0:DK],
                in0=psv.rearrange("p (h d) -> p h d", h=4),
                in1=bv_bc.rearrange("p (h d) -> p h d", h=4),
                op=ADD,
            )

        def proj_compute(ld):
            """QKV projections for a loaded task (compact form)."""
            qproj = proj.tile([P, 2, S], bf16, tag="qproj", name="qproj")
            kproj = proj.tile([P, 2, S], bf16, tag="kproj", name="kproj")
            for eo in range(2):
                qk_proj(ld, (qproj, kproj), eo)
            for m in range(8):
                v_proj(ld, m)
            return qproj, kproj, ld[10], ld[6]

        def outproj_block(t, h, xsb, wo_sb, tail=False):
            """RAW per-head output projection + ship to DRAM. At the tail
            the evacuations alternate DVE/ACT and each chunk ships as soon
            as it is cast, shortening the serial epilogue."""
            osb = obuf.tile([P, 2, 2, CHUNK], bf16, tag="osb", name="osb")
            for mo in range(2):
                pso = [
                    psP.tile([P, CHUNK], f32, tag="psp", name="pso")
                    for _ in range(2)
                ]
                for n in range(2):
                    nc.tensor.matmul(
                        pso[n],
                        lhsT=wo_sb[0:DK, h, 128 * mo : 128 * mo + 128],
                        rhs=xsb[0:DK, n, :],
                        start=True,
                        stop=True,
                    )
                if tail:
                    for n in range(2):
                        if n == 0:
                            nc.scalar.activation(
                                out=osb[:, mo, n, :],
                                in_=pso[n],
                                func=mybir.ActivationFunctionType.Copy,
                            )
                        else:
                            nc.vector.tensor_copy(out=osb[:, mo, n, :], in_=pso[n])
                        q = nc.sync if n == 0 else nc.gpsimd
                        q.dma_start(
                            out=out_d[t, h, mo, :, CHUNK * n : CHUNK * (n + 1)],
                            in_=osb[:, mo, n, :],
                        )
                else:
                    for n in range(2):
                        nc.vector.tensor_copy(out=osb[:, mo, n, :], in_=pso[n])
                    nc.gpsimd.dma_start(out=out_d[t, h, mo], in_=osb[:, mo])

        # Warm the PE p-state during the initial input-DMA wait: small
        # matmuls on a memset tile ramp the clock before real work lands.
        warm = wbuf.tile([P, P], bf16, tag="warm", name="warm")
        nc.gpsimd.memset(warm, 0.0)
        for w in range(24):
            psw = psP.tile([P, 64], f32, tag="psp", name="psw")
            nc.tensor.matmul(psw, lhsT=warm, rhs=warm[:, 0:64], start=True, stop=True)

        ld = load_task(0, spread=True)
        qproj0 = proj.tile([P, 2, S], bf16, tag="qproj", name="qproj")
        kproj0 = proj.tile([P, 2, S], bf16, tag="kproj", name="kproj")
        qk_proj(ld, (qproj0, kproj0), 0)   # eo=1 deferred into the h0/h1 loop
        v_proj(ld, 0)
        v_proj(ld, 1)
        state = (qproj0, kproj0, ld[10], ld[6])
        pending = None                       # (t, h, xsb, wo_sb) awaiting outproj
        for t in range(n_tasks):
            qproj, kproj, vsb, wo_sb = state
            ld_next = None
            for h in range(4):
                e, r0 = h // 2, 64 * (h % 2)
                psx = [
                    psX.tile([P, CHUNK], f32, tag="psx", name="psx")
                    for _ in range(2)
                ]
                for m in range(8):           # Sk tiles
                    pss = psS.tile([P, 2 * CHUNK], f32, tag="pss", name="pss")
                    for n in range(2):
                        nc.tensor.matmul(
                            pss[:, CHUNK * n : CHUNK * (n + 1)],
                            lhsT=kproj[r0 : r0 + 64, e, 128 * m : 128 * m + 128],
                            rhs=qproj[r0 : r0 + 64, e, CHUNK * n : CHUNK * (n + 1)],
                            start=True,
                            stop=True,
                        )
                    pt = ptbuf.tile([P, 2 * CHUNK], bf16, tag="pt", name="pt")
                    nc.scalar.activation(out=pt, in_=pss, func=EXP, scale=0.125)
                    if t == 0 and h == 0 and m < 6:
                        v_proj(ld, m + 2)    # ride the exp latency window
                    if t == 0 and h == 1 and m == 1:
                        qk_proj(ld, (qproj, kproj), 1)
                    if m == 1:
                        # deferred work rides the exp latency window
                        if pending is not None:
                            outproj_block(*pending)
                            pending = None
                        if h == 0 and t + 1 < n_tasks:
                            ld_next = load_task(t + 1)
                        if h == 3 and t + 1 < n_tasks:
                            next_state = proj_compute(ld_next)
                    for n in range(2):
                        nc.tensor.matmul(
                            psx[n][0 : DK + 1, :],
                            lhsT=vsb[:, m, h, :],
                            rhs=pt[:, CHUNK * n : CHUNK * (n + 1)],
                            start=(m == 0),
                            stop=(m == 7),
                        )
                xsb = xnbuf.tile([P, 2, CHUNK], bf16, tag="xsb", name="xsb")
                # split the evacuation across ACT and DVE so the PSUM banks
                # free before the next head's first AV matmul needs them
                nc.scalar.activation(
                    out=xsb[0 : DK + 1, 0, :],
                    in_=psx[0][0 : DK + 1, :],
                    func=mybir.ActivationFunctionType.Copy,
                )
                nc.vector.tensor_copy(
                    out=xsb[0 : DK + 1, 1, :], in_=psx[1][0 : DK + 1, :]
                )
                nc.gpsimd.dma_start(out=den_d[t, h], in_=xsb[DK : DK + 1, :, :])
                pending = (t, h, xsb, wo_sb)
            if t + 1 < n_tasks:
                state = next_state
        outproj_block(*pending, tail=True)

    nc.finalize()
    return nc


_cache = {}


def _get_nc():
    if "nc" not in _cache:
        _cache["nc"] = build_nc()
    return _cache["nc"]


def _tasks_of(c):
    return [NT * c + j for j in range(NT)]


def make_in_maps(query, key, value, Wq, bq, Wk, bk, Wv, bv, Wo, bo):
    import ml_dtypes

    in_dt = ml_dtypes.bfloat16
    f = np.float32
    q = np.asarray(query, f).reshape(NPAIR, S, E)
    k = np.asarray(key, f).reshape(NPAIR, S, E)
    v = np.asarray(value, f).reshape(NPAIR, S, E)
    qT = np.ascontiguousarray(q.transpose(0, 2, 1))
    kT = np.ascontiguousarray(k.transpose(0, 2, 1))
    vT = np.ascontiguousarray(v.transpose(0, 2, 1))
    Wq_, Wk_, Wv_, Wo_ = (np.asarray(w, f) for w in (Wq, Wk, Wv, Wo))
    bq_, bk_, bv_ = (np.asarray(b, f) for b in (bq, bk, bv))

    in_maps = []
    for c in range(NCORES):
        ts = _tasks_of(c)
        pairs = [t // 2 for t in ts]
        sls = [slice(F * (t % 2), F * (t % 2) + F) for t in ts]
        in_maps.append(
            {
                "ones": np.ones((P, DK), ml_dtypes.bfloat16),
                "qT": np.ascontiguousarray(qT[pairs]).astype(in_dt),
                "kT": np.ascontiguousarray(kT[pairs]).astype(in_dt),
                "vT": np.ascontiguousarray(vT[pairs]).astype(in_dt),
                "wq": np.ascontiguousarray(np.stack([Wq_[:, s] for s in sls])).astype(in_dt),
                "wk": np.ascontiguousarray(np.stack([Wk_[:, s] for s in sls])).astype(in_dt),
                "wv": np.ascontiguousarray(np.stack([Wv_[:, s] for s in sls])).astype(in_dt),
                "wo": np.ascontiguousarray(np.stack([Wo_[s, :].reshape(4, DK, OUTD) for s in sls])).astype(in_dt),
                "bq": np.stack([bq_[s] for s in sls]),
                "bk": np.stack([bk_[s] for s in sls]),
                "bv": np.stack([bv_[s] for s in sls]),
            }
        )
    return in_maps


def assemble_output(results, bo):
    out = np.zeros((NPAIR, S, OUTD), np.float32)
    for c in range(NCORES):
        o = np.asarray(results[c]["out"], np.float32)     # [NT, 4, 2, 128, S]
        den = np.asarray(results[c]["den"], np.float32)   # [NT, 4, S]
        for j, t in enumerate(_tasks_of(c)):
            x = o[j].reshape(4, OUTD, S) / den[j][:, None, :]
            out[t // 2] += x.sum(0).T
    out += np.asarray(bo, np.float32)
    return out.reshape(B, A, S, OUTD)


def kernel(query, key, value, Wq, bq, Wk, bk, Wv, bv, Wo, bo):
    import time

    in_maps = make_in_maps(query, key, value, Wq, bq, Wk, bk, Wv, bv, Wo, bo)
    last_err = None
    for _ in range(3):  # the device occasionally reports a transient
        try:            # NRT_EXEC_UNIT_UNRECOVERABLE on a fresh load; retry
            res = run_bass_kernel_spmd(
                _get_nc(), in_maps, core_ids=list(range(NCORES))
            )
            out = assemble_output(res.results, bo)
            if np.isfinite(out).all():
                return out
            last_err = RuntimeError("non-finite output")
        except Exception as e:  # noqa: BLE001
            last_err = e
        time.sleep(2)
    raise last_err
